# revision 21
# baseline (speedup 1.0000x reference)
"""Trainium2 Bass kernel for CNCAgg (weight-MLP + bmm aggregation + final 1x1 conv).

v4 strategy (8 cores, data-parallel over B=32, NO collectives):
  - Per core: 4 batches as 2 pairs. WeightNet MLP runs pair-packed with
    block-diagonal weights: one matmul computes both batches of a pair
    (halves the PE stream vs per-batch quadrant packing).
  - L3 emits transposed wgt (n on partitions, [w_b_even | w_b_odd] on free)
    in fp8 (scale A folded into w3/b3; ReLU is positively homogeneous).
  - feature is pre-transposed/quantized on host to fp8 e4m3 (4.2 MB/core).
  - bmm = fp8 x fp8 with perf_mode=DoubleRow (2 n-chunks per matmul).
  - agg is shuffled to cw-major fp8 (agg_s[p, kc, b]).
  - Final conv weight wf' (512 x 16384, BN scale + 1/N + scale F folded in)
    is REPLICATED per core in fp8 e4m3 (8.4 MB/core), fully SBUF-resident;
    the conv is a 64-step DoubleRow PSUM accumulation
    psO[4, 512] += agg_s[:, kcpair, :]^T @ wf[kcpair].
  - Epilogue: psO * 1/(A*F) + bias, ReLU (one DVE op + one ACT op).
  - No cross-core communication: each core writes its own (4, 512) output;
    host concatenates.  Max core time is independent of SPMD launch stagger.
"""

import os
import sys

sys.path.insert(0, "/opt/trn_rl_repo")

KSTAGE = int(os.environ.get("KSTAGE", "4"))

import numpy as np
import ml_dtypes

import concourse.bass as bass
from concourse import bacc
import concourse.mybir as mybir
from concourse.bass import ds, ts
from concourse.tile import TileContext
from concourse.bass_utils import run_bass_kernel_spmd

# ---------------------------------------------------------------- constants
B, N, C, OUT, W = 32, 4096, 256, 512, 64
EPS = 1e-5
NCORES = 8
BLOC = B // NCORES            # 4 batches per core
NPAIR = BLOC // 2             # 2 batch pairs per core
KCW = C * W                   # 16384 contraction dim of final conv
NKC = KCW // 128              # 128 cw-chunks of 128
NCH = N // 128                # 32 n-chunks of 128 per batch
FTI = N // 1024               # 4 feature tiles per batch (1024 pts each)
WFG = 4                       # kc-chunks per wf DMA tile
NWFT = NKC // WFG             # 32 wf DMA tiles

F32 = mybir.dt.float32
BF16 = mybir.dt.bfloat16
F8 = mybir.dt.float8e4
NPBF = ml_dtypes.bfloat16
NPF8 = ml_dtypes.float8_e4m3
RELU = mybir.ActivationFunctionType.Relu
ALU = mybir.AluOpType
DR = mybir.MatmulPerfMode.DoubleRow


def build_bass():
    nc = bacc.Bacc("TRN2", target_bir_lowering=False, debug=True,
                   num_devices=NCORES)

    # per-core inputs
    # x3p rows: 0-2 b0, 3-5 b1, 6-8 b2, 9-11 b3 (xyz^T)
    x3_d = nc.dram_tensor("x3p", [12, N], BF16, kind="ExternalInput")
    # featT[b, i, p, jj, c] = feature^T[b, n=1024*i+128*jj+p, c] (fp8)
    ft_d = nc.dram_tensor("featT", [BLOC, FTI, 128, 8, C], F8,
                          kind="ExternalInput")
    w1_d = nc.dram_tensor("w1blk", [6, 128], BF16, kind="ExternalInput")
    w2_d = nc.dram_tensor("w2blk", [128, 128], BF16, kind="ExternalInput")
    w3_d = nc.dram_tensor("w3blk", [128, 128], BF16, kind="ExternalInput")
    b1_d = nc.dram_tensor("b1", [128, 1], F32, kind="ExternalInput")
    b2_d = nc.dram_tensor("b2", [128, 1], F32, kind="ExternalInput")
    b3_d = nc.dram_tensor("b3rep", [128, 4, 128], F32, kind="ExternalInput")
    # wfT[t, p, j, o] = wf'^T[cw_lin = 128*(4t+j) + p, o] (fp8, full)
    wf_d = nc.dram_tensor("wfT", [NWFT, 128, WFG, OUT], F8,
                          kind="ExternalInput")
    bf_d = nc.dram_tensor("bfrep", [BLOC, OUT], F32, kind="ExternalInput")
    sc_d = nc.dram_tensor("scAF", [BLOC, 1], F32, kind="ExternalInput")
    out_d = nc.dram_tensor("out", [BLOC, OUT], F32, kind="ExternalOutput")
    agdbg_d = nc.dram_tensor("aggdbg", [128, NKC, BLOC], F32,
                             kind="ExternalOutput") if KSTAGE >= 90 else None

    with TileContext(nc) as tc:
        with (
            tc.tile_pool(name="const", bufs=1) as cpool,
            tc.tile_pool(name="hbuf", bufs=1) as hpool,
            tc.tile_pool(name="wgt", bufs=2) as wpool,
            tc.tile_pool(name="feat", bufs=16) as fpool,
            tc.tile_pool(name="wfin", bufs=NWFT) as wfpool,
            tc.tile_pool(name="osb", bufs=1) as opool,
            tc.tile_pool(name="ph", bufs=2, space="PSUM") as pph,
            tc.tile_pool(name="pw", bufs=1, space="PSUM") as ppw,
            tc.tile_pool(name="pa", bufs=1, space="PSUM") as ppa,
            tc.tile_pool(name="pf", bufs=1, space="PSUM") as ppf,
        ):
            # ---- constants; w1blk + x3 first (L1-critical)
            w1t = cpool.tile([6, 128], BF16, tag="w1t")
            nc.sync.dma_start(out=w1t[:], in_=w1_d[:])
            x3a = cpool.tile([6, N], BF16, tag="x3a")
            nc.sync.dma_start(out=x3a[:], in_=x3_d[ds(0, 6), :])
            x3b = cpool.tile([6, N], BF16, tag="x3b")
            nc.sync.dma_start(out=x3b[:], in_=x3_d[ds(6, 6), :])
            w2t = cpool.tile([128, 128], BF16, tag="w2t")
            nc.scalar.dma_start(out=w2t[:], in_=w2_d[:])
            w3t = cpool.tile([128, 128], BF16, tag="w3t")
            nc.scalar.dma_start(out=w3t[:], in_=w3_d[:])
            b1t = cpool.tile([128, 1], F32, tag="b1")
            nc.scalar.dma_start(out=b1t[:], in_=b1_d[:])
            b2t = cpool.tile([128, 1], F32, tag="b2")
            nc.scalar.dma_start(out=b2t[:], in_=b2_d[:])
            b3t = cpool.tile([128, 4, 128], F32, tag="b3")
            nc.scalar.dma_start(out=b3t[:], in_=b3_d[:])
            bft = cpool.tile([BLOC, OUT], F32, tag="bf")
            nc.scalar.dma_start(out=bft[:], in_=bf_d[:])
            sct = cpool.tile([BLOC, 1], F32, tag="sc")
            nc.scalar.dma_start(out=sct[:], in_=sc_d[:])
            # agg_s[p, kc, b]: agg[cw_lin = 128*kc + p, batch b] (fp8).
            # b-dim padded to 16 so the DoubleRow LDWEIGHTS kc-step is
            # 16-byte aligned ('s3_lw_dual_fp8_restrictions').
            agg_s = cpool.tile([128, NKC, 16], F8, tag="aggs")
            nc.gpsimd.memset(agg_s[:], 0.0)
            zeros = cpool.tile([128, 2, 256], BF16, tag="zeros")
            nc.vector.memset(zeros[:], 0.0)

            # ---- feature tiles: (128, 8, C) fp8, FTI per batch.  All on the
            # sync queue, strictly BEFORE the wf stream: nothing except DMAs
            # queues on sync, so trigger backpressure blocks no compute, and
            # featT gets full HBM bandwidth before wf starts.
            ft_tiles = {}

            def load_ft(b, i):
                ft = fpool.tile([128, 8, C], F8, tag="ft", name=f"ft{b}_{i}")
                nc.sync.dma_start(out=ft[:], in_=ft_d[b, i])
                ft_tiles[(b, i)] = ft

            for b in range(BLOC):
                for i in range(FTI):
                    load_ft(b, i)

            # ---- wf tiles (replicated fp8 conv weight), behind featT
            wf_tiles = []

            def load_wf(t):
                wt = wfpool.tile([128, WFG, OUT], F8, tag="wf", name=f"wf{t}")
                nc.sync.dma_start(out=wt[:], in_=wf_d[t])
                wf_tiles.append(wt)

            for t in range(NWFT):
                load_wf(t)

            # ---- L1: 3 -> 64, pair-packed block-diagonal
            # pair g rhs rows 6g..6g+5; out psum = [b_even(64) | b_odd(64)]
            # h tiles split in n-halves so L2/L3 can start at half-way.
            NH = N // 2
            h1 = [[None, None], [None, None]]
            h2 = [[None, None], [None, None]]
            for g in range(NPAIR):
                for hf in range(2):
                    h1[g][hf] = hpool.tile([128, NH], BF16,
                                           tag=f"h1_{g}{hf}",
                                           name=f"h1_{g}{hf}")
                    h2[g][hf] = hpool.tile([128, NH], BF16,
                                           tag=f"h2_{g}{hf}",
                                           name=f"h2_{g}{hf}")
            FCH = 512
            NFC = N // FCH
            for i in range(NFC):
                hf, io = i // (NFC // 2), (i % (NFC // 2)) * FCH
                psA = pph.tile([128, FCH], F32, tag="hpsA")
                psB = pph.tile([128, FCH], F32, tag="hpsB")
                nc.tensor.matmul(psA[:], lhsT=w1t[:],
                                 rhs=x3a[:, ds(i * FCH, FCH)],
                                 start=True, stop=True)
                nc.tensor.matmul(psB[:], lhsT=w1t[:],
                                 rhs=x3b[:, ds(i * FCH, FCH)],
                                 start=True, stop=True)
                nc.scalar.activation(
                    h1[0][hf][:, ds(io, FCH)], psA[:], RELU, bias=b1t[:]
                )
                nc.vector.scalar_tensor_tensor(
                    h1[1][hf][:, ds(io, FCH)],
                    in0=psB[:], scalar=b1t[:], in1=zeros[:],
                    op0=ALU.add, op1=ALU.max,
                )

            # ---- L2: 64 -> 64, pair-packed block-diagonal
            for i in range(NFC):
                hf, io = i // (NFC // 2), (i % (NFC // 2)) * FCH
                psA = pph.tile([128, FCH], F32, tag="hpsA")
                psB = pph.tile([128, FCH], F32, tag="hpsB")
                nc.tensor.matmul(psA[:], lhsT=w2t[:],
                                 rhs=h1[0][hf][:, ds(io, FCH)],
                                 start=True, stop=True)
                nc.tensor.matmul(psB[:], lhsT=w2t[:],
                                 rhs=h1[1][hf][:, ds(io, FCH)],
                                 start=True, stop=True)
                nc.vector.scalar_tensor_tensor(
                    h2[0][hf][:, ds(io, FCH)],
                    in0=psA[:], scalar=b2t[:], in1=zeros[:],
                    op0=ALU.add, op1=ALU.max,
                )
                nc.scalar.activation(
                    h2[1][hf][:, ds(io, FCH)], psB[:], RELU, bias=b2t[:]
                )

            # ---- L3 (transposed out): per pair, wgt[g] = (n x [w_e|w_o]) fp8
            wgt_tiles = [None] * NPAIR

            def l3_group(g, j):
                """4 n-chunks (4j .. 4j+4) of pair g's transposed L3."""
                if wgt_tiles[g] is None:
                    wgt_tiles[g] = wpool.tile([128, NCH, 128], F8, tag="wgt",
                                              name=f"wgt{g}")
                wgt = wgt_tiles[g]
                pwg = ppw.tile([128, 4, 128], F32, tag="wps")
                for jj in range(4):
                    ch = 4 * j + jj
                    nc.tensor.matmul(
                        pwg[:, jj, :],
                        lhsT=h2[g][ch // 16][:, ds((ch % 16) * 128, 128)],
                        rhs=w3t[:],
                        start=True, stop=True,
                    )
                nc.vector.tensor_add(pwg[:], pwg[:], b3t[:])
                nc.scalar.activation(wgt[:, ts(j, 4), :], pwg[:], RELU)

            # ---- bmm for pair g, fp8 DoubleRow over n-chunk pairs.
            # Column tiling is incompatible with dual-fp8, so the two
            # batches of a pair run sequentially through one PSUM bank.
            def bmm_pair(g, interleave=None):
                for b in (2 * g, 2 * g + 1):
                    pa = ppa.tile([W, C], F32, tag="aps", name=f"pa{b}")
                    for i in range(FTI):
                        if (b, i) not in ft_tiles:
                            load_ft(b, i)
                        ftile = ft_tiles[(b, i)]
                        wgt = wgt_tiles[g]
                        for j2 in range(4):
                            chp = 4 * i + j2     # n-chunk-pair index 0..15
                            nc.tensor.matmul(
                                pa[:],
                                lhsT=wgt[:, ds(8 * i + 2 * j2, 2),
                                         ds(64 * (b % 2), W)],
                                rhs=ftile[:, ds(2 * j2, 2), :],
                                perf_mode=DR,
                                start=(chp == 0), stop=(chp == FTI * 4 - 1),
                            )
                        if interleave is not None:
                            interleave(b % 2, i)
                    # shuffle: agg_s[64*(c%2)+w, c//2, b] = pa[w, c]
                    pav = pa[:].rearrange("w (k two) -> w two k", two=2)
                    nc.vector.tensor_copy(agg_s[0:W, :, b], pav[:, 0, :])
                    nc.vector.tensor_copy(agg_s[W:128, :, b], pav[:, 1, :])

            # L3 for pair 0 up front; pair 1's L3 interleaves into bmm(0)
            for j in range(8):
                l3_group(0, j)

            def inter0(bb, step):
                # compute pair-1 L3 inside bmm(0): 1 group per (b, i) block
                l3_group(1, 4 * bb + step)

            bmm_pair(0, interleave=inter0)
            bmm_pair(1)

            if KSTAGE >= 90:
                agf = opool.tile([128, NKC, BLOC], F32, tag="agf")
                nc.vector.tensor_copy(agf[:], agg_s[:, :, 0:BLOC])
                nc.sync.dma_start(out=agdbg_d[:], in_=agf[:])

            # ---- final conv (DoubleRow over kc pairs), two interleaved
            # accumulation chains in separate PSUM banks so LDWEIGHTS of one
            # chain hides under the other chain's matmul stream.
            psO = [ppf.tile([BLOC, OUT], F32, tag=f"fps{c}", name=f"fps{c}")
                   for c in range(2)]
            NKP = NKC // 2
            for t in range(NWFT):
                wt = wf_tiles[t]
                for u in range(2):
                    kp = 2 * t + u          # kc-pair index 0..63
                    c = kp % 2
                    nc.tensor.matmul(
                        psO[c][:],
                        lhsT=agg_s[:, ds(4 * t + 2 * u, 2), ds(0, BLOC)],
                        rhs=wt[:, ds(2 * u, 2), :],
                        perf_mode=DR,
                        start=(kp < 2), stop=(kp >= NKP - 2),
                    )
            Po = opool.tile([BLOC, OUT], F32, tag="Po")
            nc.vector.scalar_tensor_tensor(
                Po[:], in0=psO[1][:], scalar=sct[:], in1=bft[:],
                op0=ALU.mult, op1=ALU.add,
            )
            Fo = opool.tile([BLOC, OUT], F32, tag="Fo")
            nc.vector.scalar_tensor_tensor(
                Fo[:], in0=psO[0][:], scalar=sct[:], in1=Po[:],
                op0=ALU.mult, op1=ALU.add,
            )
            G = opool.tile([BLOC, OUT], F32, tag="G")
            nc.scalar.activation(G[:], Fo[:], RELU)
            nc.sync.dma_start(out=out_d[:], in_=G[:])

    nc.compile()
    return nc


_NC_CACHE = None


def _get_nc():
    global _NC_CACHE
    if _NC_CACHE is None:
        _NC_CACHE = build_bass()
    return _NC_CACHE


def _fold_bn(w, b, g, be, m, v):
    """Fold eval-mode BN into conv weight/bias: y = diag(s) W x + (s*(b-m)+be)."""
    s = (g / np.sqrt(v + EPS)).astype(np.float64)
    wp = (w.astype(np.float64) * s[:, None]).astype(np.float32)
    bp = (s * (b.astype(np.float64) - m) + be).astype(np.float32)
    return wp, bp


def _blockdiag(wT, k):
    """(k, 64) -> (2k, 128) with wT on both diagonal blocks."""
    out = np.zeros((2 * k, 128), dtype=np.float32)
    out[0:k, 0:64] = wT
    out[k:2 * k, 64:128] = wT
    return out


def prep_inputs(xyz, feature, w1, b1, g1, be1, m1, v1,
                w2, b2, g2, be2, m2, v2,
                w3, b3, g3, be3, m3, v3,
                wf, bf, gf, bef, mf, vf):
    """Host-side prep: BN folding, transposes, fp8 quantization, sharding."""
    w1p, b1p = _fold_bn(w1, b1, g1, be1, m1, v1)
    w2p, b2p = _fold_bn(w2, b2, g2, be2, m2, v2)
    w3p, b3p = _fold_bn(w3, b3, g3, be3, m3, v3)
    wfp, bfp = _fold_bn(wf, bf, gf, bef, mf, vf)
    # 1/N feature scaling folded into the final conv weight
    wfp = (wfp / N).astype(np.float32)
    # fp8 scale F for wf
    kf = int(np.floor(np.log2(448.0 / np.abs(wfp).max()))) - 2
    F = np.float32(2.0 ** kf)
    # fp8 scale A for wgt/agg: estimate agg rms from a point subsample
    xs = xyz[:, ::32, :].reshape(-1, 3).T.astype(np.float32)   # (3, B*128)
    hh = np.maximum(w1p @ xs + b1p[:, None], 0.0)
    hh = np.maximum(w2p @ hh + b2p[:, None], 0.0)
    wgt_s = np.maximum(w3p @ hh + b3p[:, None], 0.0)
    agg_rms = np.sqrt(N) * float(feature[:, ::16, ::64].std()) * \
        float(wgt_s.std()) + 1e-30
    ka = int(np.round(np.log2(8.0 / agg_rms)))
    A = np.float32(2.0 ** ka)
    w3p = (w3p * A).astype(np.float32)
    b3p = (b3p * A).astype(np.float32)
    wfq = (wfp * F).astype(np.float32)

    # wfT permuted rows: cw_lin(c, w) = 128*(c//2) + 64*(c%2) + w
    cw = np.arange(KCW)
    c_idx = cw // W
    w_idx = cw % W
    cw_lin = 128 * (c_idx // 2) + 64 * (c_idx % 2) + w_idx
    wfT_perm = np.empty((KCW, OUT), dtype=np.float32)
    wfT_perm[cw_lin] = wfq.T          # row cw_lin <- wfq[:, c*64+w]
    # [t, p, j, o] = wfT_perm[128*(4t+j) + p, o]
    wfT = wfT_perm.reshape(NWFT, WFG, 128, OUT).transpose(0, 2, 1, 3)

    shared = {
        "w1blk": _blockdiag(w1p.T, 3).astype(NPBF),
        "w2blk": _blockdiag(w2p.T, 64).astype(NPBF),
        "w3blk": _blockdiag(w3p.T, 64).astype(NPBF),
        "b1": np.tile(b1p, 2).reshape(128, 1).astype(np.float32),
        "b2": np.tile(b2p, 2).reshape(128, 1).astype(np.float32),
        "b3rep": np.tile(np.tile(b3p, 2), (128, 4, 1)).astype(np.float32),
        "bfrep": np.tile(bfp, (BLOC, 1)).astype(np.float32),
        "scAF": np.full((BLOC, 1), 1.0 / (float(A) * float(F)),
                        dtype=np.float32),
        "wfT": np.ascontiguousarray(wfT).astype(NPF8),
    }
    in_maps = []
    for core in range(NCORES):
        xs = xyz[core * BLOC:(core + 1) * BLOC]        # (4, 4096, 3)
        x3p = np.ascontiguousarray(
            xs.transpose(0, 2, 1).reshape(12, N))      # rows 3b+j
        fs = feature[core * BLOC:(core + 1) * BLOC]    # (4, 256, 4096)
        ftT = fs.transpose(0, 2, 1)                    # (4, 4096, 256)
        # [b, i, p, jj, c] = featT[b, 1024*i + 128*jj + p, c]
        ftT = ftT.reshape(BLOC, FTI, 8, 128, C).transpose(0, 1, 3, 2, 4)
        in_maps.append({
            "x3p": x3p.astype(NPBF),
            "featT": np.ascontiguousarray(ftT).astype(NPF8),
            **shared,
        })
    return in_maps


def _run(inputs, trace=False):
    inputs = {k: np.asarray(v) for k, v in inputs.items()}
    nc = _get_nc()
    in_maps = prep_inputs(
        inputs["xyz"], inputs["feature"],
        inputs["w1"], inputs["b1"], inputs["g1"], inputs["be1"], inputs["m1"], inputs["v1"],
        inputs["w2"], inputs["b2"], inputs["g2"], inputs["be2"], inputs["m2"], inputs["v2"],
        inputs["w3"], inputs["b3"], inputs["g3"], inputs["be3"], inputs["m3"], inputs["v3"],
        inputs["wf"], inputs["bf"], inputs["gf"], inputs["bef"], inputs["mf"], inputs["vf"],
    )
    res = run_bass_kernel_spmd(
        nc, in_maps, core_ids=list(range(NCORES)), trace=trace,
        trace_cores=list(range(NCORES)) if trace else None,
    )
    outs = [np.asarray(res.results[i]["out"]).reshape(BLOC, OUT)
            for i in range(NCORES)]
    full = np.concatenate(outs, axis=0).astype(np.float32)             # (32, 512)
    return full.reshape(B, OUT, 1), res


def kernel(**inputs):
    return _run(inputs, trace=False)[0]


# revision 23
# speedup vs baseline: 1.2011x; 1.2011x over previous
"""Trainium2 Bass kernel for CNCAgg (weight-MLP + bmm aggregation + final 1x1 conv).

v4 strategy (8 cores, data-parallel over B=32, NO collectives):
  - Per core: 4 batches as 2 pairs. WeightNet MLP runs pair-packed with
    block-diagonal weights: one matmul computes both batches of a pair
    (halves the PE stream vs per-batch quadrant packing).
  - L3 emits transposed wgt (n on partitions, [w_b_even | w_b_odd] on free)
    in fp8 (scale A folded into w3/b3; ReLU is positively homogeneous).
  - feature is pre-transposed/quantized on host to fp8 e4m3 (4.2 MB/core).
  - bmm = fp8 x fp8 with perf_mode=DoubleRow (2 n-chunks per matmul).
  - agg is shuffled to cw-major fp8 (agg_s[p, kc, b]).
  - Final conv weight wf' (512 x 16384, BN scale + 1/N + scale F folded in)
    is REPLICATED per core in fp8 e4m3 (8.4 MB/core), fully SBUF-resident;
    the conv is a 64-step DoubleRow PSUM accumulation
    psO[4, 512] += agg_s[:, kcpair, :]^T @ wf[kcpair].
  - Epilogue: psO * 1/(A*F) + bias, ReLU (one DVE op + one ACT op).
  - No cross-core communication: each core writes its own (4, 512) output;
    host concatenates.  Max core time is independent of SPMD launch stagger.
"""

import os
import sys

sys.path.insert(0, "/opt/trn_rl_repo")

KSTAGE = int(os.environ.get("KSTAGE", "4"))

import numpy as np
import ml_dtypes

import concourse.bass as bass
from concourse import bacc
import concourse.mybir as mybir
from concourse.bass import ds, ts
from concourse.tile import TileContext
from concourse.bass_utils import run_bass_kernel_spmd

# ---------------------------------------------------------------- constants
B, N, C, OUT, W = 32, 4096, 256, 512, 64
EPS = 1e-5
NCORES = 8
BLOC = B // NCORES            # 4 batches per core
NPAIR = BLOC // 2             # 2 batch pairs per core
KCW = C * W                   # 16384 contraction dim of final conv
NKC = KCW // 128              # 128 cw-chunks of 128
NCH = N // 128                # 32 n-chunks of 128 per batch
FTI = N // 1024               # 4 feature tiles per batch (1024 pts each)
WFG = 4                       # kc-chunks per wf DMA tile
NWFT = NKC // WFG             # 32 wf DMA tiles

F32 = mybir.dt.float32
BF16 = mybir.dt.bfloat16
F8 = mybir.dt.float8e4
NPBF = ml_dtypes.bfloat16
NPF8 = ml_dtypes.float8_e4m3
RELU = mybir.ActivationFunctionType.Relu
ALU = mybir.AluOpType
DR = mybir.MatmulPerfMode.DoubleRow


def build_bass():
    nc = bacc.Bacc("TRN2", target_bir_lowering=False, debug=True,
                   num_devices=NCORES)

    # per-core inputs
    # x3p rows: 0-2 b0, 3-5 b1, 6-8 b2, 9-11 b3 (xyz^T)
    x3_d = nc.dram_tensor("x3p", [12, N], BF16, kind="ExternalInput")
    # featT[b, i, p, jj, c] = feature^T[b, n=1024*i+128*jj+p, c] (fp8)
    ft_d = nc.dram_tensor("featT", [BLOC, FTI, 128, 8, C], F8,
                          kind="ExternalInput")
    w1_d = nc.dram_tensor("w1blk", [6, 128], BF16, kind="ExternalInput")
    w2_d = nc.dram_tensor("w2blk", [128, 128], BF16, kind="ExternalInput")
    w3_d = nc.dram_tensor("w3blk", [128, 128], BF16, kind="ExternalInput")
    b1_d = nc.dram_tensor("b1", [128, 1], F32, kind="ExternalInput")
    b2_d = nc.dram_tensor("b2", [128, 1], F32, kind="ExternalInput")
    b3_d = nc.dram_tensor("b3rep", [128, 4, 128], F32, kind="ExternalInput")
    # wfT[t, p, j, o] = wf'^T[cw_lin = 128*(4t+j) + p, o] (fp8, full)
    wf_d = nc.dram_tensor("wfT", [NWFT, 128, WFG, OUT], F8,
                          kind="ExternalInput")
    bf_d = nc.dram_tensor("bfrep", [BLOC, OUT], F32, kind="ExternalInput")
    sc_d = nc.dram_tensor("scAF", [BLOC, 1], F32, kind="ExternalInput")
    out_d = nc.dram_tensor("out", [BLOC, OUT], F32, kind="ExternalOutput")
    agdbg_d = nc.dram_tensor("aggdbg", [128, NKC, BLOC], F32,
                             kind="ExternalOutput") if KSTAGE >= 90 else None

    with TileContext(nc) as tc:
        with (
            tc.tile_pool(name="const", bufs=1) as cpool,
            tc.tile_pool(name="hbuf", bufs=1) as hpool,
            tc.tile_pool(name="wgt", bufs=2) as wpool,
            tc.tile_pool(name="feat", bufs=16) as fpool,
            tc.tile_pool(name="wfin", bufs=NWFT) as wfpool,
            tc.tile_pool(name="osb", bufs=1) as opool,
            tc.tile_pool(name="phA", bufs=2, space="PSUM") as pphA,
            tc.tile_pool(name="phB", bufs=1, space="PSUM") as pphB,
            tc.tile_pool(name="pw", bufs=2, space="PSUM") as ppw,
            tc.tile_pool(name="pa", bufs=1, space="PSUM") as ppa,
            tc.tile_pool(name="pf", bufs=1, space="PSUM") as ppf,
        ):
            # ---- constants; w1blk + x3 first (L1-critical)
            w1t = cpool.tile([6, 128], BF16, tag="w1t")
            nc.sync.dma_start(out=w1t[:], in_=w1_d[:])
            x3a = cpool.tile([6, N], BF16, tag="x3a")
            nc.sync.dma_start(out=x3a[:], in_=x3_d[ds(0, 6), :])
            x3b = cpool.tile([6, N], BF16, tag="x3b")
            nc.sync.dma_start(out=x3b[:], in_=x3_d[ds(6, 6), :])
            w2t = cpool.tile([128, 128], BF16, tag="w2t")
            nc.scalar.dma_start(out=w2t[:], in_=w2_d[:])
            w3t = cpool.tile([128, 128], BF16, tag="w3t")
            nc.scalar.dma_start(out=w3t[:], in_=w3_d[:])
            b1t = cpool.tile([128, 1], F32, tag="b1")
            nc.scalar.dma_start(out=b1t[:], in_=b1_d[:])
            b2t = cpool.tile([128, 1], F32, tag="b2")
            nc.scalar.dma_start(out=b2t[:], in_=b2_d[:])
            b3t = cpool.tile([128, 4, 128], F32, tag="b3")
            nc.scalar.dma_start(out=b3t[:], in_=b3_d[:])
            bft = cpool.tile([BLOC, OUT], F32, tag="bf")
            nc.scalar.dma_start(out=bft[:], in_=bf_d[:])
            sct = cpool.tile([BLOC, 1], F32, tag="sc")
            nc.scalar.dma_start(out=sct[:], in_=sc_d[:])
            # agg_s[p, kc, b]: agg[cw_lin = 128*kc + p, batch b] (fp8).
            # b-dim padded to 16 so the DoubleRow LDWEIGHTS kc-step is
            # 16-byte aligned ('s3_lw_dual_fp8_restrictions').
            agg_s = cpool.tile([128, NKC, 16], F8, tag="aggs")
            nc.gpsimd.memset(agg_s[:], 0.0)
            zeros = cpool.tile([128, 2, 256], BF16, tag="zeros")
            nc.vector.memset(zeros[:], 0.0)

            # ---- feature tiles: (128, 8, C) fp8, FTI per batch.  All on the
            # sync queue, strictly BEFORE the wf stream: nothing except DMAs
            # queues on sync, so trigger backpressure blocks no compute, and
            # featT gets full HBM bandwidth before wf starts.
            ft_tiles = {}

            def load_ft(b, i):
                ft = fpool.tile([128, 8, C], F8, tag="ft", name=f"ft{b}_{i}")
                nc.sync.dma_start(out=ft[:], in_=ft_d[b, i])
                ft_tiles[(b, i)] = ft

            for b in range(BLOC):
                for i in range(FTI):
                    load_ft(b, i)

            # ---- wf tiles (replicated fp8 conv weight), behind featT
            wf_tiles = []

            def load_wf(t):
                wt = wfpool.tile([128, WFG, OUT], F8, tag="wf", name=f"wf{t}")
                nc.sync.dma_start(out=wt[:], in_=wf_d[t])
                wf_tiles.append(wt)

            for t in range(NWFT):
                load_wf(t)

            # ---- L1: 3 -> 64, pair-packed block-diagonal
            # pair g rhs rows 6g..6g+5; out psum = [b_even(64) | b_odd(64)]
            # h tiles split in n-halves so L2/L3 can start at half-way.
            NH = N // 2
            h1 = [[None, None], [None, None]]
            h2 = [[None, None], [None, None]]
            for g in range(NPAIR):
                for hf in range(2):
                    h1[g][hf] = hpool.tile([128, NH], BF16,
                                           tag=f"h1_{g}{hf}",
                                           name=f"h1_{g}{hf}")
                    h2[g][hf] = hpool.tile([128, NH], BF16,
                                           tag=f"h2_{g}{hf}",
                                           name=f"h2_{g}{hf}")
            FCH = 512
            NFC = N // FCH
            for i in range(NFC):
                hf, io = i // (NFC // 2), (i % (NFC // 2)) * FCH
                psA = pphA.tile([128, FCH], F32, tag="hpsA")
                psB = pphB.tile([128, FCH], F32, tag="hpsB")
                nc.tensor.matmul(psA[:], lhsT=w1t[:],
                                 rhs=x3a[:, ds(i * FCH, FCH)],
                                 start=True, stop=True)
                nc.tensor.matmul(psB[:], lhsT=w1t[:],
                                 rhs=x3b[:, ds(i * FCH, FCH)],
                                 start=True, stop=True)
                nc.scalar.activation(
                    h1[0][hf][:, ds(io, FCH)], psA[:], RELU, bias=b1t[:]
                )
                nc.vector.scalar_tensor_tensor(
                    h1[1][hf][:, ds(io, FCH)],
                    in0=psB[:], scalar=b1t[:], in1=zeros[:],
                    op0=ALU.add, op1=ALU.max,
                )

            # ---- L2: 64 -> 64, pair-packed block-diagonal
            for i in range(NFC):
                hf, io = i // (NFC // 2), (i % (NFC // 2)) * FCH
                psA = pphA.tile([128, FCH], F32, tag="hpsA")
                psB = pphB.tile([128, FCH], F32, tag="hpsB")
                nc.tensor.matmul(psA[:], lhsT=w2t[:],
                                 rhs=h1[0][hf][:, ds(io, FCH)],
                                 start=True, stop=True)
                nc.tensor.matmul(psB[:], lhsT=w2t[:],
                                 rhs=h1[1][hf][:, ds(io, FCH)],
                                 start=True, stop=True)
                nc.vector.scalar_tensor_tensor(
                    h2[0][hf][:, ds(io, FCH)],
                    in0=psA[:], scalar=b2t[:], in1=zeros[:],
                    op0=ALU.add, op1=ALU.max,
                )
                nc.scalar.activation(
                    h2[1][hf][:, ds(io, FCH)], psB[:], RELU, bias=b2t[:]
                )

            # ---- L3 (transposed out): per pair, wgt[g] = (n x [w_e|w_o]) fp8
            wgt_tiles = [None] * NPAIR

            def l3_group(g, j):
                """4 n-chunks (4j .. 4j+4) of pair g's transposed L3."""
                if wgt_tiles[g] is None:
                    wgt_tiles[g] = wpool.tile([128, NCH, 128], F8, tag="wgt",
                                              name=f"wgt{g}")
                wgt = wgt_tiles[g]
                pwg = ppw.tile([128, 4, 128], F32, tag="wps")
                for jj in range(4):
                    ch = 4 * j + jj
                    nc.tensor.matmul(
                        pwg[:, jj, :],
                        lhsT=h2[g][ch // 16][:, ds((ch % 16) * 128, 128)],
                        rhs=w3t[:],
                        start=True, stop=True,
                    )
                nc.vector.tensor_add(pwg[:], pwg[:], b3t[:])
                nc.scalar.activation(wgt[:, ts(j, 4), :], pwg[:], RELU)

            # ---- bmm for pair g, fp8 DoubleRow over n-chunk pairs.
            # Column tiling is incompatible with dual-fp8, so the two
            # batches of a pair run sequentially through one PSUM bank.
            def bmm_pair(g, interleave=None):
                for b in (2 * g, 2 * g + 1):
                    pa = ppa.tile([W, C], F32, tag="aps", name=f"pa{b}")
                    for i in range(FTI):
                        if (b, i) not in ft_tiles:
                            load_ft(b, i)
                        ftile = ft_tiles[(b, i)]
                        wgt = wgt_tiles[g]
                        for j2 in range(4):
                            chp = 4 * i + j2     # n-chunk-pair index 0..15
                            nc.tensor.matmul(
                                pa[:],
                                lhsT=wgt[:, ds(8 * i + 2 * j2, 2),
                                         ds(64 * (b % 2), W)],
                                rhs=ftile[:, ds(2 * j2, 2), :],
                                perf_mode=DR,
                                start=(chp == 0), stop=(chp == FTI * 4 - 1),
                            )
                        if interleave is not None:
                            interleave(b % 2, i)
                    # shuffle: agg_s[64*(c%2)+w, c//2, b] = pa[w, c]
                    pav = pa[:].rearrange("w (k two) -> w two k", two=2)
                    nc.vector.tensor_copy(agg_s[0:W, :, b], pav[:, 0, :])
                    nc.vector.tensor_copy(agg_s[W:128, :, b], pav[:, 1, :])

            # L3 for pair 0 up front; pair 1's L3 interleaves into bmm(0)
            for j in range(8):
                l3_group(0, j)

            def inter0(bb, step):
                # compute pair-1 L3 inside bmm(0): 1 group per (b, i) block
                l3_group(1, 4 * bb + step)

            bmm_pair(0, interleave=inter0)
            bmm_pair(1)

            if KSTAGE >= 90:
                agf = opool.tile([128, NKC, BLOC], F32, tag="agf")
                nc.vector.tensor_copy(agf[:], agg_s[:, :, 0:BLOC])
                nc.sync.dma_start(out=agdbg_d[:], in_=agf[:])

            # ---- final conv (DoubleRow over kc pairs), two interleaved
            # accumulation chains in separate PSUM banks so LDWEIGHTS of one
            # chain hides under the other chain's matmul stream.
            psO = [ppf.tile([BLOC, OUT], F32, tag=f"fps{c}", name=f"fps{c}")
                   for c in range(2)]
            NKP = NKC // 2
            for t in range(NWFT):
                wt = wf_tiles[t]
                for u in range(2):
                    kp = 2 * t + u          # kc-pair index 0..63
                    c = kp % 2
                    nc.tensor.matmul(
                        psO[c][:],
                        lhsT=agg_s[:, ds(4 * t + 2 * u, 2), ds(0, BLOC)],
                        rhs=wt[:, ds(2 * u, 2), :],
                        perf_mode=DR,
                        start=(kp < 2), stop=(kp >= NKP - 2),
                    )
            Po = opool.tile([BLOC, OUT], F32, tag="Po")
            nc.vector.scalar_tensor_tensor(
                Po[:], in0=psO[1][:], scalar=sct[:], in1=bft[:],
                op0=ALU.mult, op1=ALU.add,
            )
            Fo = opool.tile([BLOC, OUT], F32, tag="Fo")
            nc.vector.scalar_tensor_tensor(
                Fo[:], in0=psO[0][:], scalar=sct[:], in1=Po[:],
                op0=ALU.mult, op1=ALU.add,
            )
            G = opool.tile([BLOC, OUT], F32, tag="G")
            nc.scalar.activation(G[:], Fo[:], RELU)
            nc.sync.dma_start(out=out_d[:], in_=G[:])

    nc.compile()
    return nc


_NC_CACHE = None


def _get_nc():
    global _NC_CACHE
    if _NC_CACHE is None:
        _NC_CACHE = build_bass()
    return _NC_CACHE


def _fold_bn(w, b, g, be, m, v):
    """Fold eval-mode BN into conv weight/bias: y = diag(s) W x + (s*(b-m)+be)."""
    s = (g / np.sqrt(v + EPS)).astype(np.float64)
    wp = (w.astype(np.float64) * s[:, None]).astype(np.float32)
    bp = (s * (b.astype(np.float64) - m) + be).astype(np.float32)
    return wp, bp


def _blockdiag(wT, k):
    """(k, 64) -> (2k, 128) with wT on both diagonal blocks."""
    out = np.zeros((2 * k, 128), dtype=np.float32)
    out[0:k, 0:64] = wT
    out[k:2 * k, 64:128] = wT
    return out


def prep_inputs(xyz, feature, w1, b1, g1, be1, m1, v1,
                w2, b2, g2, be2, m2, v2,
                w3, b3, g3, be3, m3, v3,
                wf, bf, gf, bef, mf, vf):
    """Host-side prep: BN folding, transposes, fp8 quantization, sharding."""
    w1p, b1p = _fold_bn(w1, b1, g1, be1, m1, v1)
    w2p, b2p = _fold_bn(w2, b2, g2, be2, m2, v2)
    w3p, b3p = _fold_bn(w3, b3, g3, be3, m3, v3)
    wfp, bfp = _fold_bn(wf, bf, gf, bef, mf, vf)
    # 1/N feature scaling folded into the final conv weight
    wfp = (wfp / N).astype(np.float32)
    # fp8 scale F for wf
    kf = int(np.floor(np.log2(448.0 / np.abs(wfp).max()))) - 2
    F = np.float32(2.0 ** kf)
    # fp8 scale A for wgt/agg: estimate agg rms from a point subsample
    xs = xyz[:, ::32, :].reshape(-1, 3).T.astype(np.float32)   # (3, B*128)
    hh = np.maximum(w1p @ xs + b1p[:, None], 0.0)
    hh = np.maximum(w2p @ hh + b2p[:, None], 0.0)
    wgt_s = np.maximum(w3p @ hh + b3p[:, None], 0.0)
    agg_rms = np.sqrt(N) * float(feature[:, ::16, ::64].std()) * \
        float(wgt_s.std()) + 1e-30
    ka = int(np.round(np.log2(8.0 / agg_rms)))
    A = np.float32(2.0 ** ka)
    w3p = (w3p * A).astype(np.float32)
    b3p = (b3p * A).astype(np.float32)
    wfq = (wfp * F).astype(np.float32)

    # wfT permuted rows: cw_lin(c, w) = 128*(c//2) + 64*(c%2) + w
    cw = np.arange(KCW)
    c_idx = cw // W
    w_idx = cw % W
    cw_lin = 128 * (c_idx // 2) + 64 * (c_idx % 2) + w_idx
    wfT_perm = np.empty((KCW, OUT), dtype=np.float32)
    wfT_perm[cw_lin] = wfq.T          # row cw_lin <- wfq[:, c*64+w]
    # [t, p, j, o] = wfT_perm[128*(4t+j) + p, o]
    wfT = wfT_perm.reshape(NWFT, WFG, 128, OUT).transpose(0, 2, 1, 3)

    shared = {
        "w1blk": _blockdiag(w1p.T, 3).astype(NPBF),
        "w2blk": _blockdiag(w2p.T, 64).astype(NPBF),
        "w3blk": _blockdiag(w3p.T, 64).astype(NPBF),
        "b1": np.tile(b1p, 2).reshape(128, 1).astype(np.float32),
        "b2": np.tile(b2p, 2).reshape(128, 1).astype(np.float32),
        "b3rep": np.tile(np.tile(b3p, 2), (128, 4, 1)).astype(np.float32),
        "bfrep": np.tile(bfp, (BLOC, 1)).astype(np.float32),
        "scAF": np.full((BLOC, 1), 1.0 / (float(A) * float(F)),
                        dtype=np.float32),
        "wfT": np.ascontiguousarray(wfT).astype(NPF8),
    }
    in_maps = []
    for core in range(NCORES):
        xs = xyz[core * BLOC:(core + 1) * BLOC]        # (4, 4096, 3)
        x3p = np.ascontiguousarray(
            xs.transpose(0, 2, 1).reshape(12, N))      # rows 3b+j
        fs = feature[core * BLOC:(core + 1) * BLOC]    # (4, 256, 4096)
        ftT = fs.transpose(0, 2, 1)                    # (4, 4096, 256)
        # [b, i, p, jj, c] = featT[b, 1024*i + 128*jj + p, c]
        ftT = ftT.reshape(BLOC, FTI, 8, 128, C).transpose(0, 1, 3, 2, 4)
        in_maps.append({
            "x3p": x3p.astype(NPBF),
            "featT": np.ascontiguousarray(ftT).astype(NPF8),
            **shared,
        })
    return in_maps


def _run(inputs, trace=False):
    inputs = {k: np.asarray(v) for k, v in inputs.items()}
    nc = _get_nc()
    in_maps = prep_inputs(
        inputs["xyz"], inputs["feature"],
        inputs["w1"], inputs["b1"], inputs["g1"], inputs["be1"], inputs["m1"], inputs["v1"],
        inputs["w2"], inputs["b2"], inputs["g2"], inputs["be2"], inputs["m2"], inputs["v2"],
        inputs["w3"], inputs["b3"], inputs["g3"], inputs["be3"], inputs["m3"], inputs["v3"],
        inputs["wf"], inputs["bf"], inputs["gf"], inputs["bef"], inputs["mf"], inputs["vf"],
    )
    res = run_bass_kernel_spmd(
        nc, in_maps, core_ids=list(range(NCORES)), trace=trace,
        trace_cores=list(range(NCORES)) if trace else None,
    )
    outs = [np.asarray(res.results[i]["out"]).reshape(BLOC, OUT)
            for i in range(NCORES)]
    full = np.concatenate(outs, axis=0).astype(np.float32)             # (32, 512)
    return full.reshape(B, OUT, 1), res


def kernel(**inputs):
    return _run(inputs, trace=False)[0]
